# revision 22
# baseline (speedup 1.0000x reference)
"""DeepseekV2 MoE layer (M=1024, H=1024, N=1024, E=16, top-6 of 8 groups x2)
on 8 Trainium2 NeuronCores.

Sharding: expert parallelism with group-aligned placement. E=16 experts in 8
groups of 2; grouped_topk keeps the top-3 groups and top_k=6 = 3*2 takes ALL
experts of those groups. Core c owns group c (experts 2c, 2c+1): the host
routes (tiny softmax over 16 logits), shards the token set per core (the
"dispatch"), and each core runs both expert MLPs on its tokens. The
shared-expert MLP is tensor-parallel over its intermediate dim (256 of 2048
per core). The host sums the per-core partials (the "combine"/unshard step).

v2 schedule (all fp16 GEMMs, fp32 PSUM):
  phase order gemm1 -> shared1 -> shared2 -> gemm2 keeps the PE warm (no
  >3.4us idle => no HAM re-throttle) and puts the small per-chunk y output
  DMAs last. All input DMAs are issued on the sync queue interleaved with the
  w1 stream so shared-expert inputs prefetch during gemm1; outputs go on the
  scalar queue. gemm2 uses stationary=w2 chunks (full 128-col utilization)
  with FD=W tokens, saving ~12k col-cycles vs FD=512. PSUM tiles are
  single-bank [128, <=512] f32 in an 8-slot ring.
"""
import sys

sys.path.insert(0, "/opt/trn_rl_repo")

import numpy as np

import concourse.mybir as mybir
import concourse.tile as tile
from concourse import bacc
from concourse.bass_utils import run_bass_kernel_spmd

P = 128
M = 1024          # tokens
H = 1024          # hidden
NI = 1024         # moe_intermediate
E = 16
N_GROUP = 8
TOPK_GROUP = 3
I_SH = 2048       # shared-expert intermediate (n_shared * moe_intermediate)
ISH_C = I_SH // 8  # per-core shared slice = 256

F32 = mybir.dt.float32
F16 = mybir.dt.float16
AF = mybir.ActivationFunctionType
MULT = mybir.AluOpType.mult

_PROGRAM_CACHE = {}


def _build_program(W):
    """SPMD program for one core; W = token capacity (multiple of 16, <=512)."""
    assert W % 16 == 0 and 0 < W <= 512
    WH = W // 2

    nc = bacc.Bacc("TRN2", target_bir_lowering=False, debug=False, num_devices=8)

    # --- per-core DRAM I/O ---
    w1s = nc.dram_tensor("w1s", [16, P, 2048], F16, kind="ExternalInput").ap()
    w2s = nc.dram_tensor("w2s", [4, P, 4096], F16, kind="ExternalInput").ap()
    xt = nc.dram_tensor("xt", [2, P, 8 * WH], F16, kind="ExternalInput").ap()
    ht = nc.dram_tensor("ht", [4, P, 2048], F16, kind="ExternalInput").ap()
    gus = nc.dram_tensor("gus", [2, P, 2048], F16, kind="ExternalInput").ap()
    dst = nc.dram_tensor("dst", [P, 2048], F16, kind="ExternalInput").ap()
    wab = nc.dram_tensor("wab", [P, 2 * W], F16, kind="ExternalInput").ap()
    yt = nc.dram_tensor("yt", [8, P, W], F16, kind="ExternalOutput").ap()
    sh = nc.dram_tensor("sh", [8, P, H], F16, kind="ExternalOutput").ap()

    with tile.TileContext(nc) as tc:
        with (
            tc.tile_pool(name="persist", bufs=1) as persist,
            tc.tile_pool(name="stream", bufs=8) as stream,
            tc.tile_pool(name="work", bufs=2) as work,
            tc.tile_pool(name="psum", bufs=8, space="PSUM") as psum,
        ):
            t_xt = persist.tile([P, 8 * W], F16, tag="xt")
            t_wab = persist.tile([P, 2 * W], F16, tag="wab")
            t_gtw = persist.tile([P, 16 * W], F16, tag="gtw")
            t_w2 = persist.tile([P, 16 * 1024], F16, tag="w2")
            t_ht = persist.tile([P, 8 * 1024], F16, tag="ht")
            t_gus = persist.tile([P, 8 * 512], F16, tag="gus")
            t_dst = persist.tile([P, 2048], F16, tag="dst")
            t_gts = persist.tile([P, 2 * 1024], F16, tag="gts")

            t_xt3 = t_xt.rearrange("p (k w) -> p k w", w=W)

            # DMA-engine warmup via gpsimd (its preamble ends ~150ns in, vs
            # ~7us for the hwdge queues): the engines ramp ~10x slow for the
            # first ~3us of activity, so kick them before the real streams.
            t_warm = persist.tile([16, 256], F16, tag="warm")
            nc.gpsimd.dma_start(out=t_warm[:], in_=ht[0][:16, :256])

            # PE warmup: HAM starts the PE at 1.2GHz and only un-throttles
            # after ~3.4us of sustained busy. Bridge the DMA wait with dummy
            # matmuls on a memset tile so real matmuls start at 2.4GHz.
            t_dmy = persist.tile([P, 512], F16, tag="dmy")
            nc.vector.memset(t_dmy[:], 0.0)
            # tapered bridge: 8 full-width dummies guarantee the >=3.4us of
            # sustained PE busy that un-throttles HAM, then 8 short ones so
            # the queue drains within ~110ns of the first real data landing
            p_dmy = psum.tile([P, 512], F32, space="PSUM", tag="acc", name="p_dmy")
            for i in range(8):
                nc.tensor.matmul(
                    p_dmy[:], t_dmy[:, :P], t_dmy[:],
                    start=(i == 0), stop=(i == 7),
                )
            for i in range(8):
                nc.tensor.matmul(
                    p_dmy[:, :P], t_dmy[:, :P], t_dmy[:, :P],
                    start=(i == 0), stop=(i == 7),
                )

            # first-MM data split across BOTH hwdge rings issued in the same
            # instant: only the first-activated ring rides the DMA warm-up
            # ramp, so at least one of (w1[0], xt) lands at full speed
            w1_tiles = [None] * 16
            w1_tiles[0] = stream.tile([P, 2048], F16, tag="w1", name="w1_0")
            nc.sync.dma_start(out=w1_tiles[0][:, :1024], in_=w1s[0, :, :1024])
            nc.scalar.dma_start(out=t_xt3[:, :, :WH], in_=xt[0])
            nc.sync.dma_start(out=w1_tiles[0][:, 1024:], in_=w1s[0, :, 1024:])
            nc.scalar.dma_start(out=t_xt3[:, :, WH:], in_=xt[1])
            # wab on scalar queue (small, needed by first gtw multiply)
            nc.scalar.dma_start(out=t_wab[:], in_=wab)

            # w1 stream issues first so its rows are never behind other
            # traffic in the per-engine DMA FIFOs; shared-expert pieces and
            # w2 go after (needed only from the shared1/gemm2 phases).
            pieces = []
            for i in range(4):
                pieces.append((t_ht[:, i * 2048:(i + 1) * 2048], ht[i]))
            for i in range(2):
                pieces.append((t_gus[:, i * 2048:(i + 1) * 2048], gus[i]))
            pieces.append((t_dst[:], dst))

            for it in range(1, 16):
                w1_tiles[it] = stream.tile(
                    [P, 2048], F16, tag="w1", name=f"w1_{it}"
                )
                nc.sync.dma_start(out=w1_tiles[it][:], in_=w1s[it])
            for dstt, src in pieces:
                nc.sync.dma_start(out=dstt, in_=src)
            for q in range(4):
                nc.sync.dma_start(
                    out=t_w2[:, q * 4096:(q + 1) * 4096], in_=w2s[q]
                )

            # --- phase A: gemm1 (routed experts gate/up + silu*up*weight) ---
            with nc.named_scope("gemm1"):
                for e in range(2):
                    for j in range(8):
                        it = e * 8 + j
                        w1t = w1_tiles[it]
                        pp = [
                            psum.tile(
                                [P, 512], F32, space="PSUM", tag="acc",
                                name=f"g1_{it}_{gu}"
                            )
                            for gu in range(2)
                        ]
                        if it == 0:
                            for th in range(2):
                                for gu in range(2):
                                    for k in range(8):
                                        nc.tensor.matmul(
                                            pp[gu][:, th * WH:(th + 1) * WH],
                                            w1t[:, (gu * 8 + k) * P:(gu * 8 + k + 1) * P],
                                            t_xt3[:, k, th * WH:(th + 1) * WH],
                                            start=(k == 0),
                                            stop=(k == 7),
                                        )
                        else:
                            for gu in range(2):
                                for k in range(8):
                                    nc.tensor.matmul(
                                        pp[gu][:, :W],
                                        w1t[:, (gu * 8 + k) * P:(gu * 8 + k + 1) * P],
                                        t_xt[:, k * W:(k + 1) * W],
                                        start=(k == 0),
                                        stop=(k == 7),
                                    )
                        sg = work.tile([P, 512], F32, tag="sg", name=f"sg_{it}")
                        nc.scalar.activation(
                            out=sg[:, :W], in_=pp[0][:, :W], func=AF.Silu
                        )
                        gt = work.tile([P, 512], F32, tag="gt", name=f"gt_{it}")
                        nc.vector.tensor_tensor(
                            out=gt[:, :W], in0=sg[:, :W], in1=pp[1][:, :W], op=MULT
                        )
                        nc.vector.tensor_tensor(
                            out=t_gtw[:, it * W:(it + 1) * W],
                            in0=gt[:, :W],
                            in1=t_wab[:, e * W:(e + 1) * W],
                            op=MULT,
                        )

            # --- phase B: shared expert gate/up (TP slice over intermediate) ---
            with nc.named_scope("shared1"):
                for mh in range(2):
                    for ip in range(2):
                        pp = []
                        for gu in range(2):
                            acc = psum.tile(
                                [P, 512], F32, space="PSUM", tag="acc",
                                name=f"s1_{mh}_{ip}_{gu}"
                            )
                            for k in range(8):
                                nc.tensor.matmul(
                                    acc[:],
                                    t_gus[:, k * 512 + gu * 256 + ip * P:
                                          k * 512 + gu * 256 + (ip + 1) * P],
                                    t_ht[:, k * M + mh * 512:k * M + (mh + 1) * 512],
                                    start=(k == 0),
                                    stop=(k == 7),
                                )
                            pp.append(acc)
                        ss = work.tile([P, 512], F32, tag="ss", name=f"ss_{mh}_{ip}")
                        nc.scalar.activation(out=ss[:], in_=pp[0][:], func=AF.Silu)
                        nc.vector.tensor_tensor(
                            out=t_gts[:, ip * M + mh * 512: ip * M + (mh + 1) * 512],
                            in0=ss[:],
                            in1=pp[1][:],
                            op=MULT,
                        )

            # --- phase C: shared expert down proj ---
            with nc.named_scope("shared2"):
                for mc in range(8):
                    so = work.tile([P, H], F16, tag="so", bufs=3, name=f"so_{mc}")
                    for oh in range(2):
                        acc = psum.tile(
                            [P, 512], F32, space="PSUM", tag="acc",
                            name=f"s2_{mc}_{oh}"
                        )
                        for ip in range(2):
                            nc.tensor.matmul(
                                acc[:],
                                t_gts[:, ip * M + mc * P: ip * M + (mc + 1) * P],
                                t_dst[:, ip * H + oh * 512: ip * H + (oh + 1) * 512],
                                start=(ip == 0),
                                stop=(ip == 1),
                            )
                        nc.scalar.copy(
                            out=so[:, oh * 512:(oh + 1) * 512], in_=acc[:]
                        )
                    # sync queue is idle by now; keeps scalar's DMA ring free
                    # for the y tail
                    nc.sync.dma_start(out=sh[mc], in_=so[:])

            # --- phase D: gemm2 (routed down proj), y in [h, token] layout ---
            # last chunk is split into two half-W groups so the final
            # copy+DMA tail after the very last matmul is half as long
            with nc.named_scope("gemm2"):
                for hc in range(8):
                    acc = psum.tile(
                        [P, 512], F32, space="PSUM", tag="acc", name=f"g2_{hc}"
                    )
                    yo = work.tile([P, W], F16, tag="yo", name=f"yo_{hc}")
                    if hc < 7:
                        for s in range(16):
                            nc.tensor.matmul(
                                acc[:, :W],
                                t_w2[:, s * 1024 + hc * P: s * 1024 + (hc + 1) * P],
                                t_gtw[:, s * W:(s + 1) * W],
                                start=(s == 0),
                                stop=(s == 15),
                            )
                        nc.vector.tensor_scalar_mul(yo[:], acc[:, :W], 1.0)
                        nc.scalar.dma_start(out=yt[hc], in_=yo[:])
                    else:
                        WQ = WH // 2
                        acc2 = psum.tile(
                            [P, 512], F32, space="PSUM", tag="acc", name="g2_7b"
                        )
                        accs = [acc, acc2]
                        for th in range(2):
                            for s in range(16):
                                nc.tensor.matmul(
                                    accs[th][:, th * WH:(th + 1) * WH],
                                    t_w2[:, s * 1024 + hc * P: s * 1024 + (hc + 1) * P],
                                    t_gtw[:, s * W + th * WH: s * W + (th + 1) * WH],
                                    start=(s == 0),
                                    stop=(s == 15),
                                )
                            if th == 0:
                                nc.vector.tensor_scalar_mul(
                                    yo[:, :WH], acc[:, :WH], 1.0
                                )
                                nc.scalar.dma_start(
                                    out=yt[hc][:, :WH], in_=yo[:, :WH]
                                )
                            else:
                                # quarter-split the very last output so the
                                # post-final-matmul copy+DMA tail is minimal
                                for q in range(2):
                                    lo = WH + q * WQ
                                    nc.vector.tensor_scalar_mul(
                                        yo[:, lo:lo + WQ], acc2[:, lo:lo + WQ], 1.0
                                    )
                                    nc.scalar.dma_start(
                                        out=yt[hc][:, lo:lo + WQ],
                                        in_=yo[:, lo:lo + WQ],
                                    )

    nc.compile()
    return nc


def _get_program(W):
    if W not in _PROGRAM_CACHE:
        _PROGRAM_CACHE[W] = _build_program(W)
    return _PROGRAM_CACHE[W]


def _route(hidden_states, gate_w):
    """Numpy replica of grouped_topk: softmax -> per-group max -> top-3 groups.
    With E=16, n_group=8, topk_group=3, top_k=6, the top-6 experts are exactly
    all experts of the top-3 groups and keep their softmax scores."""
    lg = hidden_states @ gate_w.T
    lg = lg - lg.max(axis=1, keepdims=True)
    sc = np.exp(lg)
    sc /= sc.sum(axis=1, keepdims=True)
    gsc = sc.reshape(M, N_GROUP, E // N_GROUP).max(axis=2)
    top = np.argsort(-gsc, axis=1, kind="stable")[:, :TOPK_GROUP]
    gmask = np.zeros((M, N_GROUP), bool)
    np.put_along_axis(gmask, top, True, axis=1)
    return sc.astype(np.float32), gmask


def _prep_core(c, hidden, ht4, w1, w2, sgu_t, sd_t, sc, gmask, W):
    WH = W // 2
    tok = np.nonzero(gmask[:, c])[0].astype(np.int32)
    n = len(tok)

    wab = np.zeros((2, W), np.float32)
    wab[0, :n] = sc[tok, 2 * c]
    wab[1, :n] = sc[tok, 2 * c + 1]
    wabb = np.ascontiguousarray(
        np.broadcast_to(wab.reshape(1, 2 * W), (P, 2 * W)).astype(np.float16)
    )

    xp = np.zeros((W, H), np.float32)
    xp[:n] = hidden[tok]
    # [k, p, w] -> halves [2][p, k, wh]
    xk = xp.T.astype(np.float16).reshape(8, P, W)
    xtc = np.ascontiguousarray(
        np.stack([xk[:, :, h * WH:(h + 1) * WH] for h in range(2)])
        .transpose(0, 2, 1, 3)
        .reshape(2, P, 8 * WH)
    )

    w1sc = np.empty((16, P, 2048), np.float16)
    w2l = np.empty((16, P, 1024), np.float16)
    for i, e in enumerate((2 * c, 2 * c + 1)):
        # [gu, j, q, k, p] -> [j, p, gu, k, q]
        w1sc[i * 8:(i + 1) * 8] = (
            w1[e].reshape(2, 8, P, 8, P).transpose(1, 4, 0, 3, 2)
            .reshape(8, P, 2048).astype(np.float16)
        )
        w2l[i * 8:(i + 1) * 8] = (
            np.ascontiguousarray(w2[e].T.astype(np.float16)).reshape(8, P, 1024)
        )
    w2sc = np.ascontiguousarray(
        w2l.reshape(4, 4, P, 1024).transpose(0, 2, 1, 3).reshape(4, P, 4096)
    )

    gusc = np.ascontiguousarray(
        np.concatenate(
            (
                sgu_t[:, c * ISH_C:(c + 1) * ISH_C],
                sgu_t[:, I_SH + c * ISH_C: I_SH + (c + 1) * ISH_C],
            ),
            axis=1,
        ).astype(np.float16)
    ).reshape(8, P, 512)
    gusc = np.ascontiguousarray(
        gusc.reshape(2, 4, P, 512).transpose(0, 2, 1, 3).reshape(2, P, 2048)
    )
    dstc = np.ascontiguousarray(
        sd_t[c * ISH_C:(c + 1) * ISH_C, :].astype(np.float16)
        .reshape(2, P, H).transpose(1, 0, 2).reshape(P, 2048)
    )

    return tok, {
        "w1s": w1sc,
        "w2s": w2sc,
        "xt": xtc,
        "ht": ht4,
        "gus": gusc,
        "dst": dstc,
        "wab": wabb,
    }


def _run(inputs, trace=False):
    hidden = np.ascontiguousarray(np.asarray(inputs["hidden_states"], np.float32))
    gate_w = np.asarray(inputs["gate_w"], np.float32)
    w1 = np.asarray(inputs["w1"], np.float32)
    w2 = np.asarray(inputs["w2"], np.float32)
    sgu = np.asarray(inputs["shared_gate_up"], np.float32)
    sd = np.asarray(inputs["shared_down"], np.float32)

    sc, gmask = _route(hidden, gate_w)
    counts = gmask.sum(axis=0)
    W = int(min(512, -(-int(counts.max()) // 16) * 16))
    assert counts.max() <= W, f"capacity overflow: {counts}"

    # [k, p, m] -> [4][p, kpair, m]
    ht4 = np.ascontiguousarray(
        hidden.T.astype(np.float16).reshape(4, 2, P, M)
        .transpose(0, 2, 1, 3).reshape(4, P, 2048)
    )
    sgu_t = np.ascontiguousarray(sgu.T)  # [H, 2*I_SH]
    sd_t = np.ascontiguousarray(sd.T)    # [I_SH, H]

    nc = _get_program(W)
    toks = []
    in_maps = []
    for c in range(8):
        tok, im = _prep_core(c, hidden, ht4, w1, w2, sgu_t, sd_t, sc, gmask, W)
        toks.append(tok)
        in_maps.append(im)
    res = run_bass_kernel_spmd(nc, in_maps, core_ids=list(range(8)), trace=trace)

    out = np.zeros((M, H), np.float32)
    for c in range(8):
        out += res.results[c]["sh"].reshape(M, H).astype(np.float32)
        tok = toks[c]
        yh = res.results[c]["yt"].reshape(H, W).astype(np.float32)
        out[tok] += yh[:, : len(tok)].T
    return out, res


def kernel(**inputs):
    out, _ = _run(inputs, trace=False)
    return out


# revision 23
# speedup vs baseline: 1.0472x; 1.0472x over previous
"""DeepseekV2 MoE layer (M=1024, H=1024, N=1024, E=16, top-6 of 8 groups x2)
on 8 Trainium2 NeuronCores.

Sharding: expert parallelism with group-aligned placement. E=16 experts in 8
groups of 2; grouped_topk keeps the top-3 groups and top_k=6 = 3*2 takes ALL
experts of those groups. Core c owns group c (experts 2c, 2c+1): the host
routes (tiny softmax over 16 logits), shards the token set per core (the
"dispatch"), and each core runs both expert MLPs on its tokens. The
shared-expert MLP is tensor-parallel over its intermediate dim (256 of 2048
per core). The host sums the per-core partials (the "combine"/unshard step).

v2 schedule (all fp16 GEMMs, fp32 PSUM):
  phase order gemm1 -> shared1 -> shared2 -> gemm2 keeps the PE warm (no
  >3.4us idle => no HAM re-throttle) and puts the small per-chunk y output
  DMAs last. All input DMAs are issued on the sync queue interleaved with the
  w1 stream so shared-expert inputs prefetch during gemm1; outputs go on the
  scalar queue. gemm2 uses stationary=w2 chunks (full 128-col utilization)
  with FD=W tokens, saving ~12k col-cycles vs FD=512. PSUM tiles are
  single-bank [128, <=512] f32 in an 8-slot ring.
"""
import sys

sys.path.insert(0, "/opt/trn_rl_repo")

import numpy as np

import concourse.mybir as mybir
import concourse.tile as tile
from concourse import bacc
from concourse.bass_utils import run_bass_kernel_spmd

P = 128
M = 1024          # tokens
H = 1024          # hidden
NI = 1024         # moe_intermediate
E = 16
N_GROUP = 8
TOPK_GROUP = 3
I_SH = 2048       # shared-expert intermediate (n_shared * moe_intermediate)
ISH_C = I_SH // 8  # per-core shared slice = 256

F32 = mybir.dt.float32
F16 = mybir.dt.float16
AF = mybir.ActivationFunctionType
MULT = mybir.AluOpType.mult

_PROGRAM_CACHE = {}


def _build_program(W):
    """SPMD program for one core; W = token capacity (multiple of 16, <=512)."""
    assert W % 16 == 0 and 0 < W <= 512
    WH = W // 2

    nc = bacc.Bacc("TRN2", target_bir_lowering=False, debug=False, num_devices=8)

    # --- per-core DRAM I/O ---
    w1s = nc.dram_tensor("w1s", [16, P, 2048], F16, kind="ExternalInput").ap()
    w2s = nc.dram_tensor("w2s", [4, P, 4096], F16, kind="ExternalInput").ap()
    xt = nc.dram_tensor("xt", [2, P, 8 * WH], F16, kind="ExternalInput").ap()
    ht = nc.dram_tensor("ht", [4, P, 2048], F16, kind="ExternalInput").ap()
    gus = nc.dram_tensor("gus", [2, P, 2048], F16, kind="ExternalInput").ap()
    dst = nc.dram_tensor("dst", [P, 2048], F16, kind="ExternalInput").ap()
    wab = nc.dram_tensor("wab", [P, 2 * W], F16, kind="ExternalInput").ap()
    yt = nc.dram_tensor("yt", [8, P, W], F16, kind="ExternalOutput").ap()
    sh = nc.dram_tensor("sh", [8, P, H], F16, kind="ExternalOutput").ap()

    with tile.TileContext(nc) as tc:
        with (
            tc.tile_pool(name="persist", bufs=1) as persist,
            tc.tile_pool(name="stream", bufs=8) as stream,
            tc.tile_pool(name="work", bufs=2) as work,
            tc.tile_pool(name="psum", bufs=8, space="PSUM") as psum,
        ):
            t_xt = persist.tile([P, 8 * W], F16, tag="xt")
            t_wab = persist.tile([P, 2 * W], F16, tag="wab")
            t_gtw = persist.tile([P, 16 * W], F16, tag="gtw")
            t_w2 = persist.tile([P, 16 * 1024], F16, tag="w2")
            t_ht = persist.tile([P, 8 * 1024], F16, tag="ht")
            t_gus = persist.tile([P, 8 * 512], F16, tag="gus")
            t_dst = persist.tile([P, 2048], F16, tag="dst")
            t_gts = persist.tile([P, 2 * 1024], F16, tag="gts")

            t_xt3 = t_xt.rearrange("p (k w) -> p k w", w=W)

            # DMA-engine warmup via gpsimd (its preamble ends ~150ns in, vs
            # ~7us for the hwdge queues): the engines ramp ~10x slow for the
            # first ~3us of activity, so kick them before the real streams.
            t_warm = persist.tile([16, 256], F16, tag="warm")
            nc.gpsimd.dma_start(out=t_warm[:], in_=ht[0][:16, :256])

            # PE warmup: HAM starts the PE at 1.2GHz and only un-throttles
            # after ~3.4us of sustained busy. Bridge the DMA wait with dummy
            # matmuls on a memset tile so real matmuls start at 2.4GHz.
            t_dmy = persist.tile([P, 512], F16, tag="dmy")
            nc.vector.memset(t_dmy[:], 0.0)
            p_dmy = psum.tile([P, 512], F32, space="PSUM", tag="acc", name="p_dmy")
            for i in range(16):
                nc.tensor.matmul(
                    p_dmy[:], t_dmy[:, :P], t_dmy[:],
                    start=(i == 0), stop=(i == 15),
                )

            # sync-queue input DMA schedule: first-MM data, then the w1
            # stream. (The first ~0.7MB of DMA traffic rides a ~2-3us
            # warm-up ramp no matter which ring carries it, so the first
            # matmul cannot start before ~12.5us; the dummy-matmul bridge
            # above is sized to end right then.)
            w1_tiles = [None] * 16
            w1_tiles[0] = stream.tile([P, 2048], F16, tag="w1", name="w1_0")
            nc.sync.dma_start(out=w1_tiles[0][:, :1024], in_=w1s[0, :, :1024])
            nc.sync.dma_start(out=t_xt3[:, :, :WH], in_=xt[0])
            nc.sync.dma_start(out=w1_tiles[0][:, 1024:], in_=w1s[0, :, 1024:])
            nc.sync.dma_start(out=t_xt3[:, :, WH:], in_=xt[1])
            # wab on scalar queue (small, needed by first gtw multiply)
            nc.scalar.dma_start(out=t_wab[:], in_=wab)

            # w1 stream issues first so its rows are never behind other
            # traffic in the per-engine DMA FIFOs; shared-expert pieces and
            # w2 go after (needed only from the shared1/gemm2 phases).
            pieces = []
            for i in range(4):
                pieces.append((t_ht[:, i * 2048:(i + 1) * 2048], ht[i]))
            for i in range(2):
                pieces.append((t_gus[:, i * 2048:(i + 1) * 2048], gus[i]))
            pieces.append((t_dst[:], dst))

            for it in range(1, 16):
                w1_tiles[it] = stream.tile(
                    [P, 2048], F16, tag="w1", name=f"w1_{it}"
                )
                nc.sync.dma_start(out=w1_tiles[it][:], in_=w1s[it])
            for dstt, src in pieces:
                nc.sync.dma_start(out=dstt, in_=src)
            for q in range(4):
                nc.sync.dma_start(
                    out=t_w2[:, q * 4096:(q + 1) * 4096], in_=w2s[q]
                )

            # --- phase A: gemm1 (routed experts gate/up + silu*up*weight) ---
            with nc.named_scope("gemm1"):
                for e in range(2):
                    for j in range(8):
                        it = e * 8 + j
                        w1t = w1_tiles[it]
                        pp = [
                            psum.tile(
                                [P, 512], F32, space="PSUM", tag="acc",
                                name=f"g1_{it}_{gu}"
                            )
                            for gu in range(2)
                        ]
                        if it == 0:
                            for th in range(2):
                                for gu in range(2):
                                    for k in range(8):
                                        nc.tensor.matmul(
                                            pp[gu][:, th * WH:(th + 1) * WH],
                                            w1t[:, (gu * 8 + k) * P:(gu * 8 + k + 1) * P],
                                            t_xt3[:, k, th * WH:(th + 1) * WH],
                                            start=(k == 0),
                                            stop=(k == 7),
                                        )
                        else:
                            for gu in range(2):
                                for k in range(8):
                                    nc.tensor.matmul(
                                        pp[gu][:, :W],
                                        w1t[:, (gu * 8 + k) * P:(gu * 8 + k + 1) * P],
                                        t_xt[:, k * W:(k + 1) * W],
                                        start=(k == 0),
                                        stop=(k == 7),
                                    )
                        sg = work.tile([P, 512], F32, tag="sg", name=f"sg_{it}")
                        nc.scalar.activation(
                            out=sg[:, :W], in_=pp[0][:, :W], func=AF.Silu
                        )
                        gt = work.tile([P, 512], F32, tag="gt", name=f"gt_{it}")
                        nc.vector.tensor_tensor(
                            out=gt[:, :W], in0=sg[:, :W], in1=pp[1][:, :W], op=MULT
                        )
                        nc.vector.tensor_tensor(
                            out=t_gtw[:, it * W:(it + 1) * W],
                            in0=gt[:, :W],
                            in1=t_wab[:, e * W:(e + 1) * W],
                            op=MULT,
                        )

            # --- phase B: shared expert gate/up (TP slice over intermediate) ---
            with nc.named_scope("shared1"):
                for mh in range(2):
                    for ip in range(2):
                        pp = []
                        for gu in range(2):
                            acc = psum.tile(
                                [P, 512], F32, space="PSUM", tag="acc",
                                name=f"s1_{mh}_{ip}_{gu}"
                            )
                            for k in range(8):
                                nc.tensor.matmul(
                                    acc[:],
                                    t_gus[:, k * 512 + gu * 256 + ip * P:
                                          k * 512 + gu * 256 + (ip + 1) * P],
                                    t_ht[:, k * M + mh * 512:k * M + (mh + 1) * 512],
                                    start=(k == 0),
                                    stop=(k == 7),
                                )
                            pp.append(acc)
                        ss = work.tile([P, 512], F32, tag="ss", name=f"ss_{mh}_{ip}")
                        nc.scalar.activation(out=ss[:], in_=pp[0][:], func=AF.Silu)
                        nc.vector.tensor_tensor(
                            out=t_gts[:, ip * M + mh * 512: ip * M + (mh + 1) * 512],
                            in0=ss[:],
                            in1=pp[1][:],
                            op=MULT,
                        )

            # --- phase C: shared expert down proj ---
            with nc.named_scope("shared2"):
                for mc in range(8):
                    so = work.tile([P, H], F16, tag="so", bufs=3, name=f"so_{mc}")
                    for oh in range(2):
                        acc = psum.tile(
                            [P, 512], F32, space="PSUM", tag="acc",
                            name=f"s2_{mc}_{oh}"
                        )
                        for ip in range(2):
                            nc.tensor.matmul(
                                acc[:],
                                t_gts[:, ip * M + mc * P: ip * M + (mc + 1) * P],
                                t_dst[:, ip * H + oh * 512: ip * H + (oh + 1) * 512],
                                start=(ip == 0),
                                stop=(ip == 1),
                            )
                        nc.scalar.copy(
                            out=so[:, oh * 512:(oh + 1) * 512], in_=acc[:]
                        )
                    # sync queue is idle by now; keeps scalar's DMA ring free
                    # for the y tail
                    nc.sync.dma_start(out=sh[mc], in_=so[:])

            # --- phase D: gemm2 (routed down proj), y in [h, token] layout ---
            # last chunk is split into two half-W groups so the final
            # copy+DMA tail after the very last matmul is half as long
            with nc.named_scope("gemm2"):
                for hc in range(8):
                    acc = psum.tile(
                        [P, 512], F32, space="PSUM", tag="acc", name=f"g2_{hc}"
                    )
                    yo = work.tile([P, W], F16, tag="yo", name=f"yo_{hc}")
                    if hc < 7:
                        for s in range(16):
                            nc.tensor.matmul(
                                acc[:, :W],
                                t_w2[:, s * 1024 + hc * P: s * 1024 + (hc + 1) * P],
                                t_gtw[:, s * W:(s + 1) * W],
                                start=(s == 0),
                                stop=(s == 15),
                            )
                        nc.vector.tensor_scalar_mul(yo[:], acc[:, :W], 1.0)
                        nc.scalar.dma_start(out=yt[hc], in_=yo[:])
                    else:
                        WQ = WH // 2
                        acc2 = psum.tile(
                            [P, 512], F32, space="PSUM", tag="acc", name="g2_7b"
                        )
                        accs = [acc, acc2]
                        for th in range(2):
                            for s in range(16):
                                nc.tensor.matmul(
                                    accs[th][:, th * WH:(th + 1) * WH],
                                    t_w2[:, s * 1024 + hc * P: s * 1024 + (hc + 1) * P],
                                    t_gtw[:, s * W + th * WH: s * W + (th + 1) * WH],
                                    start=(s == 0),
                                    stop=(s == 15),
                                )
                            if th == 0:
                                nc.vector.tensor_scalar_mul(
                                    yo[:, :WH], acc[:, :WH], 1.0
                                )
                                nc.scalar.dma_start(
                                    out=yt[hc][:, :WH], in_=yo[:, :WH]
                                )
                            else:
                                # quarter-split the very last output so the
                                # post-final-matmul copy+DMA tail is minimal
                                for q in range(2):
                                    lo = WH + q * WQ
                                    nc.vector.tensor_scalar_mul(
                                        yo[:, lo:lo + WQ], acc2[:, lo:lo + WQ], 1.0
                                    )
                                    nc.scalar.dma_start(
                                        out=yt[hc][:, lo:lo + WQ],
                                        in_=yo[:, lo:lo + WQ],
                                    )

    nc.compile()
    return nc


def _get_program(W):
    if W not in _PROGRAM_CACHE:
        _PROGRAM_CACHE[W] = _build_program(W)
    return _PROGRAM_CACHE[W]


def _route(hidden_states, gate_w):
    """Numpy replica of grouped_topk: softmax -> per-group max -> top-3 groups.
    With E=16, n_group=8, topk_group=3, top_k=6, the top-6 experts are exactly
    all experts of the top-3 groups and keep their softmax scores."""
    lg = hidden_states @ gate_w.T
    lg = lg - lg.max(axis=1, keepdims=True)
    sc = np.exp(lg)
    sc /= sc.sum(axis=1, keepdims=True)
    gsc = sc.reshape(M, N_GROUP, E // N_GROUP).max(axis=2)
    top = np.argsort(-gsc, axis=1, kind="stable")[:, :TOPK_GROUP]
    gmask = np.zeros((M, N_GROUP), bool)
    np.put_along_axis(gmask, top, True, axis=1)
    return sc.astype(np.float32), gmask


def _prep_core(c, hidden, ht4, w1, w2, sgu_t, sd_t, sc, gmask, W):
    WH = W // 2
    tok = np.nonzero(gmask[:, c])[0].astype(np.int32)
    n = len(tok)

    wab = np.zeros((2, W), np.float32)
    wab[0, :n] = sc[tok, 2 * c]
    wab[1, :n] = sc[tok, 2 * c + 1]
    wabb = np.ascontiguousarray(
        np.broadcast_to(wab.reshape(1, 2 * W), (P, 2 * W)).astype(np.float16)
    )

    xp = np.zeros((W, H), np.float32)
    xp[:n] = hidden[tok]
    # [k, p, w] -> halves [2][p, k, wh]
    xk = xp.T.astype(np.float16).reshape(8, P, W)
    xtc = np.ascontiguousarray(
        np.stack([xk[:, :, h * WH:(h + 1) * WH] for h in range(2)])
        .transpose(0, 2, 1, 3)
        .reshape(2, P, 8 * WH)
    )

    w1sc = np.empty((16, P, 2048), np.float16)
    w2l = np.empty((16, P, 1024), np.float16)
    for i, e in enumerate((2 * c, 2 * c + 1)):
        # [gu, j, q, k, p] -> [j, p, gu, k, q]
        w1sc[i * 8:(i + 1) * 8] = (
            w1[e].reshape(2, 8, P, 8, P).transpose(1, 4, 0, 3, 2)
            .reshape(8, P, 2048).astype(np.float16)
        )
        w2l[i * 8:(i + 1) * 8] = (
            np.ascontiguousarray(w2[e].T.astype(np.float16)).reshape(8, P, 1024)
        )
    w2sc = np.ascontiguousarray(
        w2l.reshape(4, 4, P, 1024).transpose(0, 2, 1, 3).reshape(4, P, 4096)
    )

    gusc = np.ascontiguousarray(
        np.concatenate(
            (
                sgu_t[:, c * ISH_C:(c + 1) * ISH_C],
                sgu_t[:, I_SH + c * ISH_C: I_SH + (c + 1) * ISH_C],
            ),
            axis=1,
        ).astype(np.float16)
    ).reshape(8, P, 512)
    gusc = np.ascontiguousarray(
        gusc.reshape(2, 4, P, 512).transpose(0, 2, 1, 3).reshape(2, P, 2048)
    )
    dstc = np.ascontiguousarray(
        sd_t[c * ISH_C:(c + 1) * ISH_C, :].astype(np.float16)
        .reshape(2, P, H).transpose(1, 0, 2).reshape(P, 2048)
    )

    return tok, {
        "w1s": w1sc,
        "w2s": w2sc,
        "xt": xtc,
        "ht": ht4,
        "gus": gusc,
        "dst": dstc,
        "wab": wabb,
    }


def _run(inputs, trace=False):
    hidden = np.ascontiguousarray(np.asarray(inputs["hidden_states"], np.float32))
    gate_w = np.asarray(inputs["gate_w"], np.float32)
    w1 = np.asarray(inputs["w1"], np.float32)
    w2 = np.asarray(inputs["w2"], np.float32)
    sgu = np.asarray(inputs["shared_gate_up"], np.float32)
    sd = np.asarray(inputs["shared_down"], np.float32)

    sc, gmask = _route(hidden, gate_w)
    counts = gmask.sum(axis=0)
    W = int(min(512, -(-int(counts.max()) // 16) * 16))
    assert counts.max() <= W, f"capacity overflow: {counts}"

    # [k, p, m] -> [4][p, kpair, m]
    ht4 = np.ascontiguousarray(
        hidden.T.astype(np.float16).reshape(4, 2, P, M)
        .transpose(0, 2, 1, 3).reshape(4, P, 2048)
    )
    sgu_t = np.ascontiguousarray(sgu.T)  # [H, 2*I_SH]
    sd_t = np.ascontiguousarray(sd.T)    # [I_SH, H]

    nc = _get_program(W)
    toks = []
    in_maps = []
    for c in range(8):
        tok, im = _prep_core(c, hidden, ht4, w1, w2, sgu_t, sd_t, sc, gmask, W)
        toks.append(tok)
        in_maps.append(im)
    res = run_bass_kernel_spmd(nc, in_maps, core_ids=list(range(8)), trace=trace)

    out = np.zeros((M, H), np.float32)
    for c in range(8):
        out += res.results[c]["sh"].reshape(M, H).astype(np.float32)
        tok = toks[c]
        yh = res.results[c]["yt"].reshape(H, W).astype(np.float32)
        out[tok] += yh[:, : len(tok)].T
    return out, res


def kernel(**inputs):
    out, _ = _run(inputs, trace=False)
    return out
